# revision 29
# baseline (speedup 1.0000x reference)
"""Trainium2 Bass kernel for nn_MinimalAttention (GQA attention block).

Full-input contract: kernel(**inputs) takes the unsharded numpy inputs and
returns the full output. Internally shards across 8 NeuronCores:
  - data-parallel over batch (2) x tensor-parallel over heads (4 groups of
    8 q-heads / 2 kv-heads each), per the TP sharding hint.
  - each core computes a partial [2048, 2048] output (its heads' slice of
    attn_out @ Wo rows); host sums the 4 partials per batch.

Per-core kernel structure (all matmuls bf16, fp32 PSUM accumulation):
  phase A: kT (kv heads, duplicated per q-subhead for PE row-tiling) and
           v (natural layout, augmented with a ones column so the PV matmul
           also produces the softmax denominator), streamed over 512-col
           blocks of xT.
  phase B: per seq block j: qT block, then per head-pair c:
           S^T = k_chunk.T @ q (two concurrent 64x128 PE row-tiles),
           P = exp(S^T / 8) on ScalarE (no max subtraction: |scores| < ~6),
           out^T/l = v_aug.T @ P accumulated in PSUM,
           normalize via DVE reciprocal + GPSIMD partition_broadcast;
           then the O-projection for seq block j.
"""

import os
import sys

for _p in ("/opt/trn_rl_repo", "/opt/pypackages"):
    if _p not in sys.path and os.path.isdir(_p):
        sys.path.append(_p)

import numpy as np
import ml_dtypes

import concourse.bass as bass
import concourse.bacc as bacc
import concourse.mybir as mybir
import concourse.tile as tile
from concourse.bass_utils import run_bass_kernel_spmd

HIDDEN = 2048
SEQ = 2048
NUM_HEADS = 32
NUM_KV_HEADS = 8
HEAD_DIM = 64
N_CORES = 8
TP = 4                       # head-groups
BATCH = 2
QH = NUM_HEADS // TP         # 8 local q heads -> 4 pairs
KVH = NUM_KV_HEADS // TP     # 2 local kv heads
HC = HIDDEN // 128           # 16 hidden chunks
SC = SEQ // 128              # 16 seq chunks
NJ = SEQ // 512              # 4 seq 512-blocks

BF16 = mybir.dt.bfloat16
F32 = mybir.dt.float32
EXP = mybir.ActivationFunctionType.Exp
SCALE = HEAD_DIM ** -0.5

# set by test.py to collect an NTFF profile; harness default = plain run
PROFILE = bool(os.environ.get("KERNEL_PROFILE"))
LAST_EXEC_NS = None
LAST_RESULTS = None


def _body(tc):
    nc = tc.nc
    xT = nc.declare_dram_parameter("xT", [HIDDEN, SEQ], BF16, isOutput=False)
    wq = nc.declare_dram_parameter("wq", [HIDDEN, QH * HEAD_DIM], BF16, isOutput=False)
    # wkd: kv-head columns duplicated on host -> [kv0|kv0|kv1|kv1], 256 cols
    wkd = nc.declare_dram_parameter("wkd", [HIDDEN, 2 * KVH * HEAD_DIM], BF16, isOutput=False)
    wv = nc.declare_dram_parameter("wv", [HIDDEN, KVH * HEAD_DIM], BF16, isOutput=False)
    wo = nc.declare_dram_parameter("wo", [QH * HEAD_DIM, HIDDEN], BF16, isOutput=False)
    out = nc.declare_dram_parameter("out", [SEQ, HIDDEN], F32, isOutput=True)

    consts = tc.alloc_tile_pool(name="consts", bufs=1)
    acts = tc.alloc_tile_pool(name="acts", bufs=1)

    # resident weights
    wq_sb = consts.tile([128, HC, 512], BF16)
    nc.sync.dma_start(out=wq_sb, in_=wq.rearrange("(o p) m -> p o m", p=128))
    wk_sb = consts.tile([128, HC, 256], BF16)
    nc.sync.dma_start(out=wk_sb, in_=wkd.rearrange("(o p) m -> p o m", p=128))
    wv_sb = consts.tile([128, HC, 128], BF16)
    nc.sync.dma_start(out=wv_sb, in_=wv.rearrange("(o p) m -> p o m", p=128))
    wo_sb = consts.tile([128, 4, HIDDEN], BF16)
    nc.sync.dma_start(out=wo_sb, in_=wo.rearrange("(c p) n -> p c n", p=128))

    # persistent activations
    qT = [acts.tile([128, SEQ], BF16, name=f"qT{c}") for c in range(4)]
    kTd = [acts.tile([128, SEQ], BF16, name=f"kTd{v}") for v in range(KVH)]
    # vA: [v | ones | 0*63] -> PV psum rows 0:64 = out^T, row 64 = l
    # vB: [ones | 0*63 | v] -> PV psum row 0 = l, rows 64:128 = out^T
    # (both padded to 128 weight columns so LDWEIGHTS gets fast-weight-load)
    vA = [acts.tile([128, SC, 128], BF16, name=f"vA{v}") for v in range(KVH)]
    vB = [acts.tile([128, SC, 128], BF16, name=f"vB{v}") for v in range(KVH)]
    outT = [acts.tile([128, SEQ], BF16, name=f"outT{c}") for c in range(4)]
    for v in range(KVH):
        nc.vector.memset(vA[v][:, :, 64:65], 1.0)
        nc.vector.memset(vA[v][:, :, 65:128], 0.0)
        nc.vector.memset(vB[v][:, :, 0:1], 1.0)
        nc.vector.memset(vB[v][:, :, 1:64], 0.0)
    # E broadcasts rl rows via PE: psum_r = E.T @ rl puts rl[64] (head A's
    # 1/l) on psum rows 0:64 and rl[0] (head B's) on rows 64:128. Tiles are
    # f32 (DVE can't write float32r) and bitcast to f32r for the matmul —
    # the only 4-byte dtype walrus accepts for a fused matmul here. rl rows
    # other than 0 and 64 stay zero forever.
    e_sb = acts.tile([128, 128], F32, name="e_sb")
    rl = acts.tile([128, 512], F32, name="rl")
    nc.vector.memset(e_sb, 0.0)
    nc.vector.memset(e_sb[64:65, 0:64], 1.0)
    nc.vector.memset(e_sb[0:1, 64:128], 1.0)
    nc.vector.memset(rl, 0.0)

    xT_r = xT.rearrange("(o p) s -> p o s", p=128)

    # ---- phase A: all projections (kT, v, qT) in one pass over xT ----
    with tc.tile_pool(name="xt1", bufs=2) as xt1, \
         tc.tile_pool(name="psA", bufs=2, space="PSUM") as psA:
        for j in range(NJ):
            js = slice(j * 512, (j + 1) * 512)
            xt = xt1.tile([128, HC, 512], BF16, tag="xt1")
            nc.sync.dma_start(out=xt, in_=xT_r[:, :, js])
            for mk in range(KVH):
                ps_k = psA.tile([128, 512], F32, tag="ps_k")
                for hc in range(HC):
                    nc.tensor.matmul(
                        out=ps_k,
                        lhsT=wk_sb[:, hc, mk * 128:(mk + 1) * 128],
                        rhs=xt[:, hc, :],
                        start=(hc == 0), stop=(hc == HC - 1),
                    )
                nc.vector.tensor_copy(out=kTd[mk][:, js], in_=ps_k)
            for m in range(4):
                kcg = j * 4 + m
                ps_v = psA.tile([128, 128], F32, tag="ps_v")
                for hc in range(HC):
                    nc.tensor.matmul(
                        out=ps_v,
                        lhsT=xt[:, hc, m * 128:(m + 1) * 128],
                        rhs=wv_sb[:, hc, :],
                        start=(hc == 0), stop=(hc == HC - 1),
                    )
                for v in range(KVH):
                    vs = slice(v * 64, (v + 1) * 64)
                    nc.vector.tensor_copy(out=vA[v][:, kcg, 0:64], in_=ps_v[:, vs])
                    nc.vector.tensor_copy(out=vB[v][:, kcg, 64:128], in_=ps_v[:, vs])
            for c in range(4):
                ps_q = psA.tile([128, 512], F32, tag="ps_q")
                for hc in range(HC):
                    nc.tensor.matmul(
                        out=ps_q,
                        lhsT=wq_sb[:, hc, c * 128:(c + 1) * 128],
                        rhs=xt[:, hc, :],
                        start=(hc == 0), stop=(hc == HC - 1),
                    )
                nc.vector.tensor_copy(out=qT[c][:, js], in_=ps_q)

    # ---- phase B: attention + O-proj per seq block ----
    with tc.tile_pool(name="put", bufs=2) as put_pool, \
         tc.tile_pool(name="rlb", bufs=2) as rlb_pool, \
         tc.tile_pool(name="stage", bufs=3) as stage_pool, \
         tc.tile_pool(name="pss", bufs=2, space="PSUM") as pss, \
         tc.tile_pool(name="pso", bufs=1, space="PSUM") as pso, \
         tc.tile_pool(name="psp", bufs=2, space="PSUM") as psp:

        def oproj_block(j, m):
            ms = slice((j * 4 + m) * 128, (j * 4 + m + 1) * 128)
            for n in range(4):
                ns = slice(n * 512, (n + 1) * 512)
                ps_p = psp.tile([128, 512], F32, tag="ps_p")
                for cc in range(4):
                    nc.tensor.matmul(
                        out=ps_p,
                        lhsT=outT[cc][:, ms],
                        rhs=wo_sb[:, cc, ns],
                        start=(cc == 0), stop=(cc == 3),
                    )
                st = stage_pool.tile([128, 512], F32, tag="st")
                nc.vector.tensor_copy(out=st, in_=ps_p)
                nc.sync.dma_start(out=out[ms, ns], in_=st)

        for j in range(NJ):
            js = slice(j * 512, (j + 1) * 512)
            def norm(c, ps_oA, ps_oB):
                # normalize: outT = out^T * (1/l). Copy the two l rows to
                # SBUF, PE-broadcast them to all 128 partitions (fp32 matmul
                # against the 0/1 selector E), one full-width reciprocal back
                # to SBUF, then psum*sbuf multiplies.
                nc.vector.tensor_copy(out=rl[64:65, :], in_=ps_oA[64:65, :])
                nc.vector.tensor_copy(out=rl[0:1, :], in_=ps_oB[0:1, :])
                ps_r = psp.tile([128, 512], F32, tag="ps_p")
                nc.tensor.matmul(out=ps_r, lhsT=e_sb, rhs=rl, start=True, stop=True)
                rlb = rlb_pool.tile([128, 512], F32, tag="rlb")
                nc.vector.reciprocal(out=rlb, in_=ps_r)
                nc.vector.tensor_mul(outT[c][0:64, js], ps_oA[0:64], rlb[0:64])
                nc.vector.tensor_mul(outT[c][64:128, js], ps_oB[64:128], rlb[64:128])

            pending = None
            for c in range(4):
                kv = c // 2
                # S^T + exp -> puT [128, kc, head, 512]
                puT = put_pool.tile([128, SC, 2, 512], BF16, tag="puT")
                for kc in range(SC):
                    ks = slice(kc * 128, (kc + 1) * 128)
                    ps_s = pss.tile([128, 1024], F32, tag="ps_s")
                    nc.tensor.matmul(
                        out=ps_s[:, 0:512],
                        lhsT=kTd[kv][0:64, ks],
                        rhs=qT[c][0:64, js],
                        start=True, stop=True,
                    )
                    nc.tensor.matmul(
                        out=ps_s[:, 512:1024],
                        lhsT=kTd[kv][64:128, ks],
                        rhs=qT[c][64:128, js],
                        start=True, stop=True,
                    )
                    nc.scalar.activation(
                        out=puT[:, kc].rearrange("p a b -> p (a b)"),
                        in_=ps_s,
                        func=EXP,
                        scale=SCALE,
                    )
                # fill the exp-paced PV window with PE work that has no
                # ScalarE dependency: the previous head-pair's deferred
                # normalization and one O-proj block of the previous seq
                # block (all of this block's exp inputs are already queued)
                if pending is not None:
                    norm(*pending)
                    pending = None
                if j > 0:
                    oproj_block(j - 1, c)
                # PV: out^T (+ l) accumulated over kc
                ps_oA = pso.tile([128, 512], F32, tag="ps_oA")
                ps_oB = pso.tile([128, 512], F32, tag="ps_oB")
                for kc in range(SC):
                    nc.tensor.matmul(
                        out=ps_oA,
                        lhsT=vA[kv][:, kc, :],
                        rhs=puT[:, kc, 0, :],
                        start=(kc == 0), stop=(kc == SC - 1),
                    )
                    nc.tensor.matmul(
                        out=ps_oB,
                        lhsT=vB[kv][:, kc, :],
                        rhs=puT[:, kc, 1, :],
                        start=(kc == 0), stop=(kc == SC - 1),
                    )
                pending = (c, ps_oA, ps_oB)
            norm(*pending)


        for m in range(4):
            oproj_block(NJ - 1, m)

    acts.release()
    consts.release()


_NC_CACHE = None


def _build():
    global _NC_CACHE
    if _NC_CACHE is None:
        nc = bacc.Bacc(
            "TRN2",
            target_bir_lowering=False,
            debug=False,
            enable_asserts=False,
            num_devices=N_CORES,
        )
        with tile.TileContext(nc) as tc:
            _body(tc)
        nc.compile()
        _NC_CACHE = nc
    return _NC_CACHE


def kernel(x, Wq, Wk, Wv, Wo):
    global LAST_EXEC_NS, LAST_RESULTS
    x = np.asarray(x, dtype=np.float32)
    Wq = np.asarray(Wq, dtype=np.float32)
    Wk = np.asarray(Wk, dtype=np.float32)
    Wv = np.asarray(Wv, dtype=np.float32)
    Wo = np.asarray(Wo, dtype=np.float32)
    bf = ml_dtypes.bfloat16

    in_maps = []
    for core in range(N_CORES):
        b, g = divmod(core, TP)
        qs = slice(g * QH * HEAD_DIM, (g + 1) * QH * HEAD_DIM)
        kvs = slice(g * KVH * HEAD_DIM, (g + 1) * KVH * HEAD_DIM)
        wk_g = Wk[:, kvs]
        wkd = np.concatenate(
            [wk_g[:, 0:64], wk_g[:, 0:64], wk_g[:, 64:128], wk_g[:, 64:128]], axis=1
        )
        in_maps.append({
            "xT": np.ascontiguousarray(x[b].T).astype(bf),
            "wq": Wq[:, qs].astype(bf),
            "wkd": wkd.astype(bf),
            "wv": Wv[:, kvs].astype(bf),
            "wo": np.ascontiguousarray(Wo[qs, :]).astype(bf),
        })

    nc = _build()
    res = run_bass_kernel_spmd(
        nc,
        in_maps,
        core_ids=list(range(N_CORES)),
        trace=PROFILE,
        trace_cores=list(range(N_CORES)) if PROFILE else None,
    )
    LAST_EXEC_NS = res.exec_time_ns
    LAST_RESULTS = res
    partials = [r["out"] for r in res.results]
    out = np.empty((BATCH, SEQ, HIDDEN), dtype=np.float32)
    for b in range(BATCH):
        out[b] = partials[TP * b]
        for g in range(1, TP):
            out[b] += partials[TP * b + g]
    return out
